# revision 1
# baseline (speedup 1.0000x reference)
"""LIF spiking-neuron recurrence on Trainium2 (8 NeuronCores).

Reference semantics (TAU=1, THRESH=1, f32):
    mem = 0
    for t in range(T):
        mem = mem + x[t]
        spike[t] = (mem >= 1.0) ? 1.0 : 0.0
        mem = mem * (1 - spike[t])        # hard reset

Sharding: data-parallel over the batch axis (B=128 -> 16 rows/core).
Per-core layout: the [T, 16, 16384] shard is viewed as [T, 128, 2048]
(partition-major within a timestep slab) and pre-transposed on the host
to [128, T, 2048] so each partition's DMA runs are contiguous.

Engine mapping per timestep (tile [128, 2048] f32):
    DVE : tmp = mem + x_t            (tensor_tensor add, 1x, ~2.29us)
    ACT : d = Sqrt(tmp + (-1))       (NaN iff tmp < 1; affine is exact)
    ACT : spike = Is_finite(d)       (exact 1.0/0.0, written as bf16)
    DVE : mem = (tmp < 1) * tmp      (scalar_tensor_tensor, 1x, ~2.29us)
The ACT spike route was probed exact on HW for all threshold edge
cases (ties, +-1ulp); GpSimd is kept idle (f32 elementwise there runs
~15-30x below DVE and its shared-port lock stalls DVE). Spikes are
stored as uint8 (0/1 exact, probed) cutting store traffic 4x; the
host upcasts. DMAs are HWDGE (loads on SP ring, stores on ACT ring);
loads are per-step 1MB transfers (slice-level deps let each add start
as soon as its own slice lands), stores per-group except the last
group which stores per-step to shorten the tail. Step 0 uses x_0
directly (mem starts at 0); the final step's reset is dead code.

Measured on 8 axon-tunneled trn2 cores: ~320us HW exec time
(neuron-profile, core 0), bit-exact vs the jax f32 reference.
Steady state is DVE-bound at ~4.73us/step (TT 2290ns + STT 2290ns +
dispatch); the uint8 output puts DMA (~84MB/core) below the DVE floor.
"""

import numpy as np

try:
    import concourse  # noqa: F401
except ImportError:  # pragma: no cover
    import sys

    for _p in ("/opt/trn_rl_repo", "/root/.axon_site/_ro/trn_rl_repo"):
        if _p not in sys.path:
            sys.path.insert(0, _p)

from concourse import bacc, mybir
from concourse.bass_utils import run_bass_kernel_spmd
from concourse.mybir import ActivationFunctionType as AF
from concourse.mybir import AluOpType
from concourse.tile import TileContext

T, B, D = 64, 128, 16384
NCORES = 8
BL = B // NCORES  # 16 batch rows per core
P = 128  # SBUF partitions
F = (BL * D) // P  # 2048 free elements per timestep slab
CHUNK = 4  # timesteps per DMA transfer


def build_nc(
    t_steps=T, f_free=F, chunk=CHUNK, x_bufs=4, s_bufs=3, t_bufs=4, d_bufs=1
):
    """Build + compile the per-core Bass program (identical on all cores)."""
    assert t_steps % chunk == 0
    f32 = mybir.dt.float32
    u8 = mybir.dt.uint8
    nc = bacc.Bacc(
        "TRN2", target_bir_lowering=False, debug=False, num_devices=NCORES
    )
    x_ext = nc.dram_tensor("x", [P, t_steps, f_free], f32, kind="ExternalInput")
    out_ext = nc.dram_tensor(
        "out", [P, t_steps, f_free], u8, kind="ExternalOutput"
    )
    n_groups = t_steps // chunk
    with TileContext(nc) as tc:
        with (
            tc.tile_pool(name="xp", bufs=x_bufs) as xp,
            tc.tile_pool(name="sp", bufs=s_bufs) as sp,
            tc.tile_pool(name="tp", bufs=t_bufs) as tp,
            tc.tile_pool(name="dp", bufs=d_bufs) as dp,
            tc.tile_pool(name="mp", bufs=1) as mp,
        ):
            mem = mp.tile([P, f_free], f32)
            bm1 = mp.tile([P, 1], f32, name="bm1")
            nc.vector.memset(bm1[:], -1.0)
            for g in range(n_groups):
                xt = xp.tile([P, chunk * f_free], f32, name="xt")
                xv = x_ext[:, g * chunk : (g + 1) * chunk, :]
                # per-step loads: slice-level deps let each TT start as
                # soon as its own 1MB lands instead of the whole 4MB
                for j in range(chunk):
                    nc.sync.dma_start(
                        xt[:, j * f_free : (j + 1) * f_free], xv[:, j, :]
                    )
                spk = sp.tile([P, chunk * f_free], u8, name="spk")
                for j in range(chunk):
                    t = g * chunk + j
                    xs = xt[:, j * f_free : (j + 1) * f_free]
                    ss = spk[:, j * f_free : (j + 1) * f_free]
                    if t == 0:
                        pre = xs  # mem==0: pre-reset membrane is just x_0
                    else:
                        tmp = tp.tile([P, f_free], f32, name="tmp")
                        nc.vector.tensor_tensor(
                            tmp[:], mem[:], xs, AluOpType.add
                        )
                        pre = tmp[:]
                    # spike = Is_finite(Sqrt(pre - 1)): NaN iff pre < 1
                    d = dp.tile([P, f_free], f32, name="d")
                    nc.scalar.activation(
                        d[:], pre, AF.Sqrt, bias=bm1[:], scale=1.0
                    )
                    nc.scalar.activation(
                        ss, d[:], AF.Is_finite, bias=0.0, scale=1.0
                    )
                    if t < t_steps - 1:  # last reset is dead code
                        nc.vector.scalar_tensor_tensor(
                            mem[:], pre, 1.0, pre, AluOpType.is_lt, AluOpType.mult
                        )
                    if g == n_groups - 1:
                        # per-step stores so the tail drains quickly
                        nc.scalar.dma_start(
                            out_ext[:, g * chunk + j, :], ss
                        )
                if g < n_groups - 1:
                    nc.scalar.dma_start(
                        out_ext[:, g * chunk : (g + 1) * chunk, :].rearrange(
                            "p t f -> p (t f)"
                        ),
                        spk[:],
                    )
    nc.compile()
    return nc


_cached_nc = None


def _get_nc():
    global _cached_nc
    if _cached_nc is None:
        _cached_nc = build_nc()
    return _cached_nc


def _shard(x):
    """Full [T, B, D] -> list of per-core [P, T, F] contiguous arrays."""
    in_maps = []
    for c in range(NCORES):
        xc = x[:, c * BL : (c + 1) * BL, :].reshape(T, P, F).transpose(1, 0, 2)
        in_maps.append({"x": np.ascontiguousarray(xc)})
    return in_maps


def _gather(results):
    """Per-core [P, T, F] uint8 outputs -> full [T, B, D] f32 (exact)."""
    outs = [
        np.asarray(results[c]["out"])
        .astype(np.float32)
        .transpose(1, 0, 2)
        .reshape(T, BL, D)
        for c in range(NCORES)
    ]
    return np.concatenate(outs, axis=1)


def run(x, trace=False, **kw):
    """Run on the 8 NeuronCores; returns (output, BassKernelResults)."""
    x = np.ascontiguousarray(np.asarray(x, dtype=np.float32))
    assert x.shape == (T, B, D), x.shape
    nc = _get_nc()
    res = run_bass_kernel_spmd(
        nc, _shard(x), core_ids=list(range(NCORES)), trace=trace, **kw
    )
    return _gather(res.results), res


def kernel(x: np.ndarray) -> np.ndarray:
    out, _ = run(x)
    return out



# revision 3
# speedup vs baseline: 1.1174x; 1.1174x over previous
"""LIF spiking-neuron recurrence on Trainium2 (8 NeuronCores) — v4.

Reference semantics (TAU=1, THRESH=1, f32):
    mem = 0
    for t in range(T):
        mem = mem + x[t]
        spike[t] = (mem >= 1.0) ? 1.0 : 0.0
        mem = mem * (1 - spike[t])        # hard reset

v4 = v3 (fused custom-DVE LIF step) + PE output packing.

The loop-carried step is ONE custom DVE instruction per timestep via a
state re-encoding (m̂ stores pre-reset membrane, minus 4096 on spiking
steps; rounding on that branch is harmless since it decodes to 0):

    m   = m̂ * (m̂ > -2048)          # decode: spiked last step -> 0
    pre = m + x_t                    # exact f32 add (matches reference)
    m̂' = pre - (pre >= 1) * 4096    # encode: spike flag in value range

ACT (one step behind, off the chain) computes enc = Sign(m̂ + 2048)
∈ {+1, -1} as bf16 (-1 = spike).  The TENSOR engine then packs 8
timesteps into one int16 per element: psum += (4^j · I) @ enc_j for
j = 0..7; since each digit is ±1 the sum Σ s_j 4^j (|·| ≤ 21845, odd,
exact in f32/int16) uniquely decodes greedily from the top digit.
This cuts output traffic 4x (16 MiB -> 4 MiB per core); the kernel is
DMA-bound at the per-core HBM rate (~368 GB/s), so bytes ARE time.

Sharding: data-parallel over batch (B=128 -> 16 rows/core); per-core
x viewed as [128, T, 2048] (partition-major within a timestep slab).
"""

import numpy as np

try:
    import concourse  # noqa: F401
except ImportError:  # pragma: no cover
    import sys

    for _p in ("/opt/trn_rl_repo", "/root/.axon_site/_ro/trn_rl_repo"):
        if _p not in sys.path:
            sys.path.insert(0, _p)

from concourse import bacc, mybir
from concourse import dve_ops as _dve_ops
from concourse.bass_utils import run_bass_kernel_spmd
from concourse.dve_spec import C0, C1, One, Spec, Src0, Src1, lower
from concourse.dve_uop import DveOpSpec
from concourse.mybir import ActivationFunctionType as AF
from concourse.tile import TileContext

T, B, D = 64, 128, 16384
NCORES = 8
BL = B // NCORES  # 16 batch rows per core
P = 128
F = (BL * D) // P  # 2048 free elements per timestep slab
G = 8  # timesteps packed per int16 output group
NG = T // G
NS = 4  # 512-col PSUM bank slices
SL = F // NS

DEC = -2048.0  # decode threshold
ENC = 4096.0  # encode offset

f32 = mybir.dt.float32
bf16 = mybir.dt.bfloat16
s16 = mybir.dt.int16
f8e4 = mybir.dt.float8e4
u8 = mybir.dt.uint8


def _lif_reference(in0, in1, s0, s1, imm2):
    m = in0 * (in0 > s0)
    pre = (m + in1).astype(np.float32)
    return (pre - (pre >= 1.0) * np.float32(s1)).astype(np.float32)


def _register_lif_op():
    """Register the fused LIF-step op with the custom-DVE table (documented
    extension point: define a DveOp and append to dve_ops.OPS)."""
    name = "LIF_STEP_ANT"
    for op in _dve_ops.OPS:
        if op.name == name:
            return op
    m = Src0 * (Src0 > C0)
    pre = m + Src1
    spec = Spec(body=pre - (pre >= One) * C1, reference=_lif_reference)
    row = _dve_ops._CUSTOM_DVE_ROW_BASE + len(_dve_ops.OPS)
    assert row < 0x20
    sha = {
        ver: DveOpSpec(
            name=name, opcode=row, uops=lower(spec, ver=ver), rd1_en=True
        ).sha(ver)
        for ver in ("v3", "v4")
    }
    op = _dve_ops.DveOp(name, spec, subdim=False, uops_sha=sha)
    _dve_ops.OPS.append(op)
    _dve_ops._SUB_OPCODE_FOR_NAME[name] = row
    _dve_ops.CUSTOM_DVE_SPECS[name] = spec
    return op


LIF_OP = _register_lif_op()


def build_nc(t_steps=T, x_bufs=8):
    nc = bacc.Bacc(
        "TRN2", target_bir_lowering=False, debug=False, num_devices=NCORES
    )
    x_ext = nc.dram_tensor("x", [P, t_steps, F], f32, kind="ExternalInput")
    w_ext = nc.dram_tensor("wpack", [P, G, P], bf16, kind="ExternalInput")
    out_ext = nc.dram_tensor("out", [P, NG, F], s16, kind="ExternalOutput")
    with TileContext(nc) as tc:
        with (
            tc.tile_pool(name="xp", bufs=x_bufs) as xp,
            tc.tile_pool(name="ep", bufs=5) as ep,
            tc.tile_pool(name="op", bufs=2) as op_pool,
            tc.tile_pool(name="mp", bufs=1) as mp,
            tc.tile_pool(name="pp", bufs=2, space="PSUM") as pp,
        ):
            b2048 = mp.tile([P, 1], f32, name="b2048")
            nc.vector.memset(b2048[:], 2048.0)
            wpk = mp.tile([P, G, P], bf16, name="wpk")
            nc.sync.dma_start(wpk[:], w_ext[:])
            # 3 state buffers: ACT(t) reads mh[(t+1)%3], which DVE rewrites
            # only at t+3 -> two full steps of slack before ACT can stall DVE
            NMH = 3
            mh = [mp.tile([P, F], f32, name=f"mh{i}") for i in range(NMH)]
            nc.vector.memset(mh[0][:], 0.0)

            pk = None
            for t in range(t_steps):
                j = t % G
                g = t // G
                if j == 0:
                    pk = pp.tile([P, F], f32, name="pk")
                xt = xp.tile([P, F], f32, name="xt")
                nc.sync.dma_start(xt[:], x_ext[:, t, :])
                src, dst = mh[t % NMH], mh[(t + 1) % NMH]
                nc.vector._custom_dve(
                    LIF_OP, out=dst[:], in0=src[:], in1=xt[:], s0=DEC, s1=ENC
                )
                enc = ep.tile([P, F], bf16, name="enc")
                nc.scalar.activation(
                    enc[:], dst[:], AF.Sign, bias=b2048[:], scale=1.0
                )
                for s in range(NS):
                    nc.tensor.matmul(
                        pk[:, s * SL : (s + 1) * SL],
                        wpk[:, j, :],
                        enc[:, s * SL : (s + 1) * SL],
                        start=(j == 0),
                        stop=(j == G - 1),
                    )
                if j == G - 1:
                    pko = op_pool.tile([P, F], s16, name="pko")
                    nc.scalar.activation(pko[:], pk[:], AF.Copy, bias=0.0)
                    nc.scalar.dma_start(out_ext[:, g, :], pko[:])
    nc.compile()
    return nc


_cached_nc = None


def _get_nc():
    global _cached_nc
    if _cached_nc is None:
        _cached_nc = build_nc()
    return _cached_nc


def _shard(x):
    import ml_dtypes

    eye = np.eye(P, dtype=np.float32)
    wpk = np.stack([eye * float(4**j) for j in range(G)], axis=1)
    wpk_bf16 = wpk.astype(ml_dtypes.bfloat16)  # 4^j and 0 are exact in bf16
    in_maps = []
    for c in range(NCORES):
        xc = x[:, c * BL : (c + 1) * BL, :].reshape(T, P, F).transpose(1, 0, 2)
        in_maps.append({"x": np.ascontiguousarray(xc), "wpack": wpk_bf16})
    return in_maps


def _gather(results):
    """Decode [P, NG, F] int16 packed base-4 signed digits to spikes.

    val = sum_j s_j 4^j with s_j in {+1 (no spike), -1 (spike)}; val is
    odd and |sum_{i<j} s_i 4^i| < 4^j, so the sign of the remainder
    gives s_j greedily from the top digit.
    """
    outs = []
    for c in range(NCORES):
        vals = np.asarray(results[c]["out"])  # [P, NG, F] int16
        v = vals.astype(np.int32)
        sp = np.empty((P, T, F), dtype=np.float32)
        for j in range(G - 1, -1, -1):
            d = np.where(v > 0, np.int32(1), np.int32(-1))
            for g in range(NG):
                sp[:, g * G + j, :] = (d[:, g, :] < 0).astype(np.float32)
            v = v - (d << (2 * j))
        outs.append(sp.transpose(1, 0, 2).reshape(T, BL, D))
    return np.concatenate(outs, axis=1)


def run(x, trace=False, **kw):
    x = np.ascontiguousarray(np.asarray(x, dtype=np.float32))
    assert x.shape == (T, B, D), x.shape
    nc = _get_nc()
    res = run_bass_kernel_spmd(
        nc, _shard(x), core_ids=list(range(NCORES)), trace=trace, **kw
    )
    return _gather(res.results), res


def kernel(x: np.ndarray) -> np.ndarray:
    out, _ = run(x)
    return out
